# revision 44
# baseline (speedup 1.0000x reference)
"""Trainium2 Bass kernel for nn_NeuralMemory (B=4, N=1024, D=128, DEPTH=4).

Sharding: 8 cores, core c handles batch b = c//2, retrieval half h = c%2.
Both cores of a pair compute the full store phase (grad sums over all
1024 tokens are order-invariant); each retrieves its own 512 tokens.
No cross-core communication.

Host-side prep (free): seq ships pre-transposed feature-major bf16
(tile 0 packed with w0eff so the forward's first matmul has its bytes
first, tile 1 packed with the other store weights), token-major
pair-packed bf16 (stm, lhsT chunks for M), and the own-half
feature-major f32 (stf) for the retrieval query path. Weights ship
precomposed: w0eff = Wk@w0, backward transposes w1T..w3T, the 2/D loss
scale folded into w3/Wv, Wv negated so V accumulates as -V into the H3
bank.

Store phase is bf16: silu writes bf16 activations straight to SBUF
(consumed by both the next layer matmul and the dW transposes), backward
multiplies write bf16 g's directly, dW transposes/matmuls are bf16.
Forward is emitted layer-major so the two token tiles pipeline on the
in-order PE queue. The dW accumulator uses one PSUM bank with four
independent per-slot accumulation chains; dW1+M run on the critical
tail while dW3/dW2 (whose only deadline is the late u3/u2 adds) are
pinned after M. K is never materialized (H0 = S @ (Wk w0)) and the
retrieval first layer is rewritten
  X1 = X0 @ w0 + (X0 @ Wk^T) @ M,  M = S^T @ G0.

ACT-table discipline: all forward Silus before any Derivative_silu,
and a dummy Silu reloads the silu table before retrieval. H2 is held
in PSUM for sp2; H1/H0 are *recomputed* into the hold banks right
before their dsilus (PE has slack there), so the forward needs no
H evictions at all and the pre-backward DVE window stays clear.
GPSIMD cannot touch PSUM on TRN2 and f32r matmul operands must come
from a rounding producer, so: all PSUM evictions live on DVE (gtm1 on
ACT's post-dsilu window, pt on ACT's post-reload gap), Pool only does
SBUF->SBUF work (f32->f32r casts of wfa/stf, u3 bf16 cast). The
relaxed-deadline gtm3/gtm2 transposes and the dW3/dW2 accumulations
run after the M path, interleaved with the retrieval by consumer
deadline (u2 before px3, u3 before po). p_g1 routes via tr banks,
p_g0 via the mm pool; px1 runs as four late matmuls in one mm bank.
The final layer runs bf16 (r3/u3b) to dodge the 4x f32r penalty on
128-col matmuls.
"""

import numpy as np
import ml_dtypes

import concourse.bass as bass
import concourse.mybir as mybir
import concourse.tile as tile
from concourse import bacc
from concourse.bass import ts
from concourse.bass_utils import run_bass_kernel_spmd
from concourse.masks import make_identity

B, N, D = 4, 1024, 128
DEPTH = 4
NCORES = 8
NT = 512            # tokens retrieved per core (half a batch)
TT = 512            # store-phase token tile
NTI = N // TT
NCHUNK = N // 128
RH = 256            # retrieval sub-tile

f32 = mybir.dt.float32
f32r = mybir.dt.float32r
bf16 = mybir.dt.bfloat16

AF = mybir.ActivationFunctionType
ALU = mybir.AluOpType

BF_NP = ml_dtypes.bfloat16

# pk2 (bf16) layout: [w1, w2, w3*(2/D), -Wv*(2/D), w1T, w2T, w3T, st_t1]
PK_W1, PK_W2, PK_W3S, PK_WVS, PK_W1T, PK_W2T, PK_W3T = range(7)
# wfa (f32) slots
FA_WQ, FA_W0, FA_WKT = range(3)
# wfb (f32) slots
FB_W1, FB_W2, FB_W3 = range(3)


def _build_program(reps=1):
    nc = bacc.Bacc(
        "TRN2",
        target_bir_lowering=False,
        debug=False,
        enable_asserts=False,
        num_devices=NCORES,
    )

    pk1 = nc.dram_tensor("pk1", [D, D + TT], bf16, kind="ExternalInput").ap()
    pk1b = nc.dram_tensor("pk1b", [D, TT], bf16, kind="ExternalInput").ap()
    pk2 = nc.dram_tensor("pk2", [D, 7 * D], bf16, kind="ExternalInput").ap()
    wfa_dr = nc.dram_tensor("wfa", [D, 3 * D], f32, kind="ExternalInput").ap()
    stf_dr = nc.dram_tensor("stf", [D, NT], f32, kind="ExternalInput").ap()
    stm_dr = nc.dram_tensor("stm", [N // 2, 2 * D], bf16, kind="ExternalInput").ap()
    wfb_dr = nc.dram_tensor("wfb", [D, 3 * D], f32, kind="ExternalInput").ap()
    out_dr = nc.dram_tensor("out", [NT, D], f32, kind="ExternalOutput").ap()

    with tile.TileContext(nc) as tc:
        for _ in range(reps):
            _emit(tc, pk1, pk1b, pk2, wfa_dr, stf_dr, stm_dr, wfb_dr, out_dr)

    nc.compile()
    return nc


def _emit(tc, pk1, pk1b, pk2, wfa_dr, stf_dr, stm_dr, wfb_dr, out_dr):
    nc = tc.nc
    from contextlib import ExitStack

    from concourse.tile_rust import add_dep_helper as _dep  # type: ignore

    with ExitStack() as ctx:
        consts = ctx.enter_context(tc.tile_pool(name="consts", bufs=1))
        big = ctx.enter_context(tc.tile_pool(name="big", bufs=1))
        # PSUM banks: mm(3) + hold(2) + tr(2) + dw(1) = 8
        pp = ctx.enter_context(tc.tile_pool(name="pp", bufs=1, space="PSUM"))

        def pmm(name, w=TT, dt=f32):
            return pp.tile([128, w], dt, tag="mm", bufs=3, name=name)

        def phold(name, w=TT, dt=f32):
            return pp.tile([128, w], dt, tag="hold", bufs=2, name=name)

        def ptr(name):
            return pp.tile([128, 512], bf16, tag="tr", bufs=2, name=name)

        # tiny scratch silu pulls the first ACT table load off the
        # critical path (runs during the DMAs)
        # PE warm-up: establish the p-state ramp clock early
        wupa = consts.tile([128, 128], f32r, tag="wupa")
        nc.gpsimd.memset(wupa[:].bitcast(f32), 0.0)
        scr = consts.tile([128, 1], f32, tag="scr")
        scr2 = consts.tile([128, 1], f32, tag="scr2")
        nc.gpsimd.memset(scr[:], 0.0)
        nc.scalar.activation(scr2[:], scr[:], AF.Silu)

        wupp = pmm("wupp")
        for k in range(3):
            nc.tensor.matmul(
                wupp[:, 0:128], wupa[:], wupa[:],
                skip_group_check=True,
            )

        ident = consts.tile([128, 128], f32, tag="ident")
        make_identity(nc, ident)
        ident_b = consts.tile([128, 128], bf16, tag="ident_b")
        nc.gpsimd.tensor_copy(ident_b[:], ident[:])

        # ---- DMAs ordered by need ----
        w0st0 = big.tile([128, D + TT], bf16, tag="w0st0")
        nc.sync.dma_start(w0st0[:], pk1)
        w0eff = w0st0[:, 0:D]
        st1 = big.tile([128, TT], bf16, tag="st1")
        nc.sync.dma_start(st1[:], pk1b)
        wbs = big.tile([128, 7 * D], bf16, tag="wbs")
        nc.sync.dma_start(wbs[:], pk2)

        def stt(t):          # feature-major S^T, tile t
            return w0st0[:, D : D + TT] if t == 0 else st1[:]

        def wbt(i):          # store-phase weight slots
            return wbs[:, ts(i, D)]

        wfa_f = big.tile([128, 3, D], f32, tag="wfa_f")
        nc.sync.dma_start(wfa_f[:], wfa_dr.rearrange("p (c d) -> p c d", d=D))
        wfa = big.tile([128, 3, D], f32r, tag="wfa")
        nc.gpsimd.tensor_copy(wfa[:], wfa_f[:])
        stf_f = big.tile([128, NT], f32, tag="stf_f")
        nc.sync.dma_start(stf_f[:], stf_dr)
        stf = big.tile([128, NT], f32r, tag="stf")
        nc.gpsimd.tensor_copy(stf[:], stf_f[:])
        stm = big.tile([128, NCHUNK // 2, 2 * D], bf16, tag="stm")
        nc.sync.dma_start(stm[:], stm_dr.rearrange("(c p) d -> p c d", p=128))
        wfb = big.tile([128, 3, D], f32r, tag="wfb")
        nc.sync.dma_start(
            wfb[:].bitcast(f32), wfb_dr.rearrange("p (c d) -> p c d", d=D)
        )

        # persistent SBUF activations (bf16, feature-major)
        a1 = big.tile([128, N], bf16, tag="a1")
        a2 = big.tile([128, N], bf16, tag="a2")
        a3 = big.tile([128, N], bf16, tag="a3")
        g3b = big.tile([128, N], bf16, tag="g3b")
        g2 = big.tile([128, N], bf16, tag="g2")
        g1 = big.tile([128, N], bf16, tag="g1")
        g0 = big.tile([128, N], bf16, tag="g0")
        sp2 = big.tile([128, N], bf16, tag="sp2")
        sp1 = big.tile([128, N], bf16, tag="sp1")
        sp0 = big.tile([128, N], bf16, tag="sp0")
        x0 = big.tile([128, NT], f32r, tag="x0")
        pt = big.tile([128, NT], f32r, tag="pt")
        atm1 = big.tile([128, N], bf16, tag="atm1")
        atm2 = big.tile([128, N], bf16, tag="atm2")
        atm3 = big.tile([128, N], bf16, tag="atm3")
        gtm0 = big.tile([128, N], bf16, tag="gtm0")
        gtm1 = big.tile([128, N], bf16, tag="gtm1")
        gtm2 = big.tile([128, N], bf16, tag="gtm2")
        gtm3 = big.tile([128, N], bf16, tag="gtm3")

        silu_insts = []
        dsilu_insts = []

        def transpose_half(src, dst, h, name, eng, pool="tr"):
            if pool == "tr":
                p = ptr(name)
            elif pool == "hold":
                p = phold(name, 512, bf16)
            else:
                p = pmm(name, 512, bf16)
            for j in range(4):
                c = h * 4 + j
                nc.tensor.matmul(
                    p[:, ts(j, 128)], src[:, ts(c, 128)], ident_b[:],
                    is_transpose=True,
                )
            if eng == "pool":
                return nc.gpsimd.tensor_copy(dst[:, ts(h, 512)], p[:])
            elif eng == "act":
                return nc.scalar.activation(dst[:, ts(h, 512)], p[:], AF.Copy)
            else:
                return nc.vector.tensor_copy(dst[:, ts(h, 512)], p[:])

        # ---- forward, layer-major so the tiles pipeline on PE ----
        h0s, h1s, h2s = [], [], []
        for t in range(NTI):
            h0 = pmm(f"h0_{t}")
            nc.tensor.matmul(h0[:], w0eff, stt(t))
            h0s.append(h0)
        for t in range(NTI):
            sl = ts(t, TT)
            silu_insts.append(nc.scalar.activation(a1[:, sl], h0s[t][:], AF.Silu))
        for t in range(NTI):
            h1 = pmm(f"h1_{t}")
            nc.tensor.matmul(h1[:], wbt(PK_W1), a1[:, ts(t, TT)])
            h1s.append(h1)
        for t in range(NTI):
            sl = ts(t, TT)
            silu_insts.append(nc.scalar.activation(a2[:, sl], h1s[t][:], AF.Silu))
        # a1/a2 transposes + query path fill the PE idle slots; atm2
        # evictions precede x0's in the DVE queue (their data lands first)
        for h in range(2):
            transpose_half(a1, atm1, h, f"p_a1{h}", "dve")
        for t in range(NTI):
            h2 = phold(f"h2_{t}")
            nc.tensor.matmul(h2[:], wbt(PK_W2), a2[:, ts(t, TT)])
            h2s.append(h2)
        for t in range(NTI):
            silu_insts.append(
                nc.scalar.activation(a3[:, ts(t, TT)], h2s[t][:], AF.Silu)
            )
        for h in range(2):
            transpose_half(a2, atm2, h, f"p_a2{h}", "dve")
        px = pmm("p_x0")
        nc.tensor.matmul(px[:], wfa[:, FA_WQ], stf[:])
        nc.vector.tensor_copy(x0[:], px[:])

        # g3 = (2/D)(H3 - V) accumulated in one bank (scales folded)
        h3s = []
        for t in range(NTI):
            sl = ts(t, TT)
            h3 = pmm(f"h3_{t}")
            nc.tensor.matmul(h3[:], wbt(PK_W3S), a3[:, sl], start=True, stop=False)
            nc.tensor.matmul(h3[:], wbt(PK_WVS), stt(t), start=False, stop=True)
            h3s.append(h3)
        for t in range(NTI):
            nc.vector.tensor_copy(g3b[:, ts(t, TT)], h3s[t][:])

        # ---- dsilu phase (single table switch after all silus) ----
        for h in range(2):
            transpose_half(a3, atm3, h, f"p_a3{h}", "dve")
        # H1/H0 are recomputed into the hold banks freed by the sp2 reads:
        # no forward hsb evictions, dsilu reads PSUM directly
        dsilu_insts.append(
            nc.scalar.activation(sp2[:, ts(0, TT)], h2s[0][:], AF.Derivative_silu)
        )
        dsilu_insts.append(
            nc.scalar.activation(sp2[:, ts(1, TT)], h2s[1][:], AF.Derivative_silu)
        )
        h1r_mms = []
        for t in range(NTI):
            hr = phold(f"h1r_{t}")
            h1r_mms.append(nc.tensor.matmul(hr[:], wbt(PK_W1), a1[:, ts(t, TT)]))
            dsilu_insts.append(
                nc.scalar.activation(sp1[:, ts(t, TT)], hr[:], AF.Derivative_silu)
            )
        for t in range(NTI):
            hr = phold(f"h0r_{t}")
            nc.tensor.matmul(hr[:], w0eff, stt(t))
            dsilu_insts.append(
                nc.scalar.activation(sp0[:, ts(t, TT)], hr[:], AF.Derivative_silu)
            )
        for di in dsilu_insts:
            _dep(di.ins, silu_insts[-1].ins, sync=False, reason="act-table order")

        # ---- backward chain; c-matmuls lead, transposes trail ----
        c2s = []
        for t in range(NTI):
            c2 = pmm(f"c2_{t}")
            mm = nc.tensor.matmul(c2[:], wbt(PK_W3T), g3b[:, ts(t, TT)])
            if t == 1:
                _dep(mm.ins, h1r_mms[0].ins, sync=False, reason="h1r first")
            c2s.append(c2)
        for t in range(NTI):
            nc.vector.tensor_mul(g2[:, ts(t, TT)], c2s[t][:], sp2[:, ts(t, TT)])
        c1s = []
        for t in range(NTI):
            c1 = pmm(f"c1_{t}")
            nc.tensor.matmul(c1[:], wbt(PK_W2T), g2[:, ts(t, TT)])
            c1s.append(c1)
        for t in range(NTI):
            nc.vector.tensor_mul(g1[:, ts(t, TT)], c1s[t][:], sp1[:, ts(t, TT)])
        c0s = []
        for t in range(NTI):
            c0 = pmm(f"c0_{t}")
            nc.tensor.matmul(c0[:], wbt(PK_W1T), g1[:, ts(t, TT)])
            c0s.append(c0)
        for t in range(NTI):
            nc.vector.tensor_mul(g0[:, ts(t, TT)], c0s[t][:], sp0[:, ts(t, TT)])
        ppt = pmm("p_pt")
        nc.tensor.matmul(ppt[:], wfa[:, FA_WKT], x0[:])
        # p_g1 through tr banks (ACT evicts them in its post-dsilu window),
        # p_g0 through the mm pool; gtm3/gtm2 (relaxed deadline) move past M
        gtm1_evicts = []
        for h in range(2):
            gtm1_evicts.append(transpose_half(g1, gtm1, h, f"p_g1{h}", "act"))
        gtm0_evicts = []
        for h in range(2):
            gtm0_evicts.append(
                transpose_half(g0, gtm0, h, f"p_g0{h}", "dve", pool="mm")
            )

        # dW accumulator: one bank, four independent per-slot chains.
        acc = pp.tile([128, 4, 128], f32, tag="dw", bufs=1, name="dwacc")

        def dw_mms(lhs_chunks, gtm, slot):
            out = []
            for c in range(NCHUNK):
                out.append(nc.tensor.matmul(
                    acc[:, slot],
                    lhs_chunks(c),
                    gtm[:, ts(c, 128)],
                    start=(c == 0),
                    stop=(c == NCHUNK - 1),
                ))
            return out

        dw_mms(lambda c: atm1[:, ts(c, 128)], gtm1, 2)
        m_mms = dw_mms(lambda c: stm[:, c // 2, ts(c % 2, 128)], gtm0, 3)

        # reload the silu table before retrieval, off the tail; gtm1 evicts
        # take ACT's window first, pt is evicted in the post-reload gap
        scr3 = consts.tile([128, 1], f32, tag="scr3")
        dummy = nc.scalar.activation(scr3[:], scr[:], AF.Silu)
        _dep(dummy.ins, dsilu_insts[-1].ins, sync=False, reason="act-table order")
        for ev in gtm1_evicts:
            _dep(dummy.ins, ev.ins, sync=False, reason="gtm1 before the reload")
        pt_cp = nc.scalar.activation(pt[:], ppt[:], AF.Copy)
        for ev in gtm1_evicts:
            _dep(pt_cp.ins, ev.ins, sync=False, reason="gtm1 feeds dW1 first")

        m_r = big.tile([128, 128], f32r, tag="m_r")
        m_r_cp = nc.vector.tensor_copy(m_r[:], acc[:, 3])
        u1 = consts.tile([D, D], f32r, tag="u1")
        u2 = consts.tile([D, D], f32r, tag="u2")
        u3 = consts.tile([D, D], f32r, tag="u3")
        u3b = consts.tile([D, D], bf16, tag="u3b")
        nc.vector.tensor_add(u1[:], acc[:, 2], wfb[:, FB_W1])

        # ---- retrieval: X1 = X0 @ w0 + P @ M, layers 2..4 ----
        # dW2/dW3 transposes+evicts interleave by consumer deadline:
        # u2 chain before px3, u3 chain before po
        r1 = big.tile([128, NT], f32r, tag="r1")
        r2 = big.tile([128, NT], f32r, tag="r2")
        r3 = big.tile([128, NT], bf16, tag="r3")
        o_tm = big.tile([128, NT // 128, 128], f32, tag="o_tm")
        out_r = out_dr.rearrange("(c p) d -> p c d", p=128)

        px1 = pmm("px1")
        for hh in range(2):
            nc.tensor.matmul(px1[:, ts(hh, RH)], wfa[:, FA_W0], x0[:, ts(hh, RH)],
                             start=(hh == 0), stop=False)
        for hh in range(2):
            nc.tensor.matmul(px1[:, ts(hh, RH)], m_r[:], pt[:, ts(hh, RH)],
                             start=False, stop=(hh == 1))
        for hh in range(2):
            nc.scalar.activation(r1[:, ts(hh, RH)], px1[:, ts(hh, RH)], AF.Silu)

        for h in range(2):
            transpose_half(g2, gtm2, h, f"p_g2{h}", "dve")
        dw2_mms = dw_mms(lambda c: atm2[:, ts(c, 128)], gtm2, 1)
        for mm in dw2_mms:
            _dep(mm.ins, m_mms[-1].ins, sync=False, reason="keep off the M path")
        nc.vector.tensor_add(u2[:], acc[:, 1], wfb[:, FB_W2])

        for hh in range(2):
            px2 = pmm(f"px2_{hh}", RH)
            nc.tensor.matmul(px2[:], u1[:], r1[:, ts(hh, RH)])
            nc.scalar.activation(r2[:, ts(hh, RH)], px2[:], AF.Silu)

        for h in range(2):
            transpose_half(g3b, gtm3, h, f"p_g3{h}", "dve")
        dw3_mms = dw_mms(lambda c: atm3[:, ts(c, 128)], gtm3, 0)
        for mm in dw3_mms:
            _dep(mm.ins, m_mms[-1].ins, sync=False, reason="keep off the M path")
        nc.vector.tensor_add(u3[:], acc[:, 0], wfb[:, FB_W3])
        nc.gpsimd.tensor_copy(u3b[:], u3[:].bitcast(f32))

        for hh in range(2):
            px3 = pmm(f"px3_{hh}", RH)
            nc.tensor.matmul(px3[:], u2[:], r2[:, ts(hh, RH)])
            nc.scalar.activation(r3[:, ts(hh, RH)], px3[:], AF.Silu)
        for hh in range(2):
            po = pmm(f"po{hh}", RH)
            for j in range(RH // 128):
                c = hh * (RH // 128) + j
                nc.tensor.matmul(
                    po[:, ts(j, 128)],
                    r3[:, ts(c, 128)],
                    u3b[:],
                    start=(j == 0),
                    stop=(j == RH // 128 - 1),
                )
            dst = o_tm[:, 2 * hh : 2 * hh + 2]
            posrc = po[:].rearrange("p (c d) -> p c d", d=128)
            if hh == 0:
                nc.scalar.activation(dst, posrc, AF.Copy)
            else:
                nc.vector.tensor_copy(dst, posrc)
        nc.sync.dma_start(out_r[:], o_tm[:])


_CACHE = {}


def _get_nc():
    if "nc" not in _CACHE:
        _CACHE["nc"] = _build_program()
    return _CACHE["nc"]


def _host_inputs(seq, w0, w1, w2, w3, wq, wkv):
    """Per-core input maps: layout/dtype packing + tiny d x d weight algebra."""
    seq = np.ascontiguousarray(np.asarray(seq, np.float32))
    w0, w1, w2, w3, wq, wkv = (
        np.asarray(x, np.float32) for x in (w0, w1, w2, w3, wq, wkv)
    )
    wk = wkv[:, :D]
    wv = wkv[:, D:]
    sc = np.float32(2.0 / D)
    w0eff = wk @ w0
    wstore = np.concatenate(
        [w1, w2, w3 * sc, -wv * sc, w1.T, w2.T, w3.T], axis=1
    )
    wfa = np.ascontiguousarray(np.concatenate([wq, w0, wk.T], axis=1))
    wfb = np.ascontiguousarray(np.concatenate([w1, w2, w3], axis=1))

    in_maps = []
    for c in range(NCORES):
        b, h = c // 2, c % 2
        s = seq[b]                                  # [N, D]
        pk1 = np.concatenate([w0eff, s[:TT].T], axis=1).astype(BF_NP)
        pk1b = np.ascontiguousarray(s[TT:].T).astype(BF_NP)
        pk2 = np.ascontiguousarray(wstore).astype(BF_NP)
        # token-major pair-packed: row j*128+p = [tok (2j)*128+p, tok (2j+1)*128+p]
        sc4 = s.reshape(NCHUNK // 2, 2, 128, D)
        stm = np.ascontiguousarray(
            sc4.transpose(0, 2, 1, 3).reshape(N // 2, 2 * D)
        ).astype(BF_NP)
        stf = np.ascontiguousarray(s[h * NT : (h + 1) * NT].T)  # [D, NT] f32
        in_maps.append(
            {"pk1": pk1, "pk1b": pk1b, "pk2": pk2, "wfa": wfa, "stf": stf,
             "stm": stm, "wfb": wfb}
        )
    return in_maps


def kernel(seq, w0, w1, w2, w3, wq, wkv):
    nc = _get_nc()
    in_maps = _host_inputs(seq, w0, w1, w2, w3, wq, wkv)
    res = run_bass_kernel_spmd(nc, in_maps, core_ids=list(range(NCORES)))
    _CACHE["last_results"] = res

    out = np.empty((B, N, D), np.float32)
    for c in range(NCORES):
        b, h = c // 2, c % 2
        out[b, h * NT : (h + 1) * NT] = res.results[c]["out"]
    return out
